# revision 36
# baseline (speedup 1.0000x reference)
"""Trainium2 Bass kernel for nn_AttentiveGatingv2 (moe_routing).

Reference computation (shapes hardcoded):
  x: [64, 96, 207, 64] -> take last 6 timesteps -> per-(b,n) token:
  z = proj(x_k); qkv = in_proj(z); 4-head attention over the 6 steps;
  out-proj; mean over steps; fc to 8 experts; softmax -> [64, 207, 8].

Host-side algebraic fusion (verified vs reference):
  W_eff = in_proj_w @ proj_w  (96x64), b_eff = in_proj_w@proj_b + in_proj_b
  (q-rows pre-scaled by 1/sqrt(8)); since mean-over-steps commutes with the
  linear out-proj/fc, post-attention collapses to
  logits = G @ (sum_j wbar_j * v_j) + g_b  with  G = fc_w@out_w/6,
  g_b = fc_w@out_b + fc_b,  wbar_j = sum_i softmax_j(scores)_ij.

Layout strategy: host pre-slices the 6 needed timesteps (1/16 of x), casts
to bf16, packs them feature-major with an appended ones-row so the single
PE matmul  qkv[tokens,96] = x_aug[65,tokens].T @ W_aug[65,96]  lands
token-major in PSUM (fp32) with bias included.  Attention math runs on
VectorE in bf16 (validated ~5e-4 rel-to-max on the final softmax output),
with 128-token tiles processed in groups of 4 so the small softmax/context
ops amortize instruction overhead; expert-logit matmuls run per pair of
tiles through one PE transpose + a block-diagonal G.  8 NeuronCores
data-parallel over batch; no cross-device communication.

Measured on trn2 (8 cores, via axon): HW exec ~58.5us/core, rel err 5.4e-4.
Progression: 160us (first correct fp32) -> 90 (drop serializing hacks) ->
73 (bf16) -> 62 (pair batching) -> 58.5 (quad batching, load stagger,
scalar-queue const loads, deeper work pool). GQ=6 measured
worse (60.4us: batched ops stall behind 6 evacuations), so GQ=4 stands.
"""

import numpy as np
import ml_dtypes

import concourse.bass as bass
import concourse.mybir as mybir
import concourse.tile as tile
from concourse.bacc import Bacc
from concourse.bass_utils import run_bass_kernel_spmd

F32 = mybir.dt.float32
BF16 = mybir.dt.bfloat16
NP_BF16 = ml_dtypes.bfloat16

# problem dims
B, T, NTOK, C = 64, 96, 207, 64
D, H, HD, K = 32, 4, 8, 6
E = 8
NCORES = 8

# per-core dims
B_SH = B // NCORES            # 8
S = B_SH * NTOK               # 1656 tokens per core
P = 128
NT = (S + P - 1) // P         # 13 tiles
S_PAD = NT * P                # 1664
CA = C + 1                    # 65: channels + ones row
E3 = 3 * D                    # 96
KK = K * K                    # 36
DA = D + 1                    # 33


def _build_module():
    nc = Bacc()

    xt = nc.dram_tensor("xt", [CA, K, S_PAD], BF16, kind="ExternalInput")
    # wa columns: q (32, pre-scaled) | k (32) | ve (32 = 4 heads x 8 experts)
    # where ve_(h,e) = G[e, 8h:8h+8] @ v_(h,:) + g_b[e]/24, so
    # logits = sum_{j,h} wbar[j,h] * ve[j,h,:] exactly (sum wbar = 24).
    wa = nc.dram_tensor("wa", [CA, E3], BF16, kind="ExternalInput")
    # out[p, t, e]: token (t*128+p); host reassembles.
    out = nc.dram_tensor("out", [P, NT, E], F32, kind="ExternalOutput")

    AF = mybir.ActivationFunctionType
    AX = mybir.AxisListType

    def apv(t, dims, extra_offset=0):
        # custom AP over tile t: keep t's partition dim, replace free dims
        return bass.AP(
            tensor=t.tensor,
            offset=t.offset + extra_offset,
            ap=[list(t.ap[0])] + [list(d) for d in dims],
        )

    KH = K * H
    # two groups, each carried to completion (through its own logits +
    # expert-softmax + store): group 0's tail overlaps group 1's compute,
    # so the end-of-kernel serial chain is only group 1's half
    GROUPS = [(0, 7), (7, 6)]

    with tile.TileContext(nc) as tc:
        with (
            tc.tile_pool(name="singles", bufs=1) as singles,
            tc.tile_pool(name="xload", bufs=1) as xload,
            tc.tile_pool(name="psum", bufs=4, space="PSUM") as psum,
        ):
            # wa on the scalar queue: overlaps the x tile-0 load on sync
            wa_sb = singles.tile([CA, E3], BF16)
            nc.scalar.dma_start(out=wa_sb, in_=wa[:, :])

            # early tiles land ASAP: tile 0 split across BOTH hw queues,
            # tiles 1-3 alternate queues; the rest in two chunks
            load_plan = [(0, 1), (1, 1), (2, 1), (3, 1), (4, 4), (8, 5)]
            xt_sb = [None] * NT
            xld = []
            for li, (t0, n) in enumerate(load_plan):
                xc_sb = xload.tile([CA, K, n * P], BF16, name=f"xc{t0}",
                                   tag=f"xc{t0}")
                if li == 0:
                    nc.sync.dma_start(out=xc_sb[:, 0:3, :],
                                      in_=xt[:, 0:3, t0 * P:(t0 + 1) * P])
                    d = nc.scalar.dma_start(
                        out=xc_sb[:, 3:K, :],
                        in_=xt[:, 3:K, t0 * P:(t0 + 1) * P])
                else:
                    eng = nc.scalar if li in (2, 5) else nc.sync
                    d = eng.dma_start(out=xc_sb,
                                      in_=xt[:, :, t0 * P:(t0 + n) * P])
                xld.append(d)
                for u in range(n):
                    xt_sb[t0 + u] = xc_sb[:, :, u * P:(u + 1) * P]

            qkv_first = []
            for gi, (t0, n) in enumerate(GROUPS):
                tmp_g = singles.tile([P, n, KK, D], BF16, name=f"tmp{gi}")
                ve_g = singles.tile([P, n, K, D], BF16, name=f"ve{gi}")
                for u in range(n):
                    t = t0 + u
                    # [P, 8, 128] = exactly 2 PSUM banks (bank-aligned slots)
                    qkv_ps = psum.tile([P, 8, 128], F32, tag="qkv_ps",
                                       bufs=4, name="qkv_ps")
                    for i in range(K):
                        mm = nc.tensor.matmul(
                            out=qkv_ps[:, i, 0:E3],
                            lhsT=xt_sb[t][:, i, :],
                            rhs=wa_sb[:, :],
                            start=True,
                            stop=True,
                        )
                        if i == 0:
                            qkv_first.append(mm)

                    # evacuate as bf16 on ScalarE; per-tile qk tile keeps the
                    # evac->tmp dependency precise
                    qk_sb = singles.tile([P, K, 2 * D], BF16, name=f"qk{t}")
                    nc.scalar.copy(out=qk_sb, in_=qkv_ps[:, 0:K, 0:2 * D])
                    nc.scalar.copy(out=ve_g[:, u],
                                   in_=qkv_ps[:, 0:K, 2 * D:E3])

                    # scores tmp[i,j,(h,c)] = q[i,(hc)] * k[j,(hc)]
                    q_ap = apv(qk_sb, [[2 * D, K], [0, K], [1, D]])
                    k_ap = apv(qk_sb, [[0, K], [2 * D, K], [1, D]], D)
                    tm_out = apv(tmp_g, [[D, KK], [1, D]], u * KK * D)
                    nc.vector.tensor_mul(tm_out, q_ap, k_ap)

                # ---- c-reduction tree batched over the group ----
                aKK = n * KK
                s1 = singles.tile([P, n, KK, H, 4], BF16, name=f"s1_{gi}")
                nc.vector.tensor_add(
                    apv(s1, [[16, aKK], [4, H], [1, 4]]),
                    apv(tmp_g, [[D, aKK], [HD, H], [1, 4]]),
                    apv(tmp_g, [[D, aKK], [HD, H], [1, 4]], 4))
                s2 = singles.tile([P, n, KK, H, 2], BF16, name=f"s2_{gi}")
                nc.vector.tensor_add(
                    apv(s2, [[8, aKK], [2, H], [1, 2]]),
                    apv(s1, [[16, aKK], [4, H], [1, 2]]),
                    apv(s1, [[16, aKK], [4, H], [1, 2]], 2))
                scores = singles.tile([P, n, KK, H], F32, name=f"sc_{gi}")
                nc.vector.tensor_add(
                    apv(scores, [[H, aKK], [1, H]]),
                    apv(s2, [[8, aKK], [2, H]]),
                    apv(s2, [[8, aKK], [2, H]], 1))
                es = singles.tile([P, n, K, K, H], BF16, name=f"es_{gi}")
                nc.scalar.activation(out=es, in_=scores, func=AF.Exp)
                nK = n * K
                zs1 = singles.tile([P, n, K, 3, H], BF16, name=f"zs1_{gi}")
                nc.vector.tensor_add(
                    apv(zs1, [[3 * H, nK], [H, 3], [1, H]]),
                    apv(es, [[K * H, nK], [H, 3], [1, H]]),
                    apv(es, [[K * H, nK], [H, 3], [1, H]], 3 * H))
                zs2 = singles.tile([P, n, K, H], BF16, name=f"zs2_{gi}")
                nc.vector.tensor_add(
                    apv(zs2, [[H, nK], [1, H]]),
                    apv(zs1, [[3 * H, nK], [1, H]]),
                    apv(zs1, [[3 * H, nK], [1, H]], H))
                zs = singles.tile([P, n, K, H], F32, name=f"zs_{gi}")
                nc.vector.tensor_add(
                    apv(zs, [[H, nK], [1, H]]),
                    apv(zs2, [[H, nK], [1, H]]),
                    apv(zs1, [[3 * H, nK], [1, H]], 2 * H))
                rs = singles.tile([P, n, K, H], F32, name=f"rs_{gi}")
                nc.vector.reciprocal_approx_fast(
                    apv(rs, [[1, nK * H]]), apv(zs, [[1, nK * H]]))
                rsb = singles.tile([P, n, K, H], BF16, name=f"rsb_{gi}")
                nc.vector.tensor_copy(
                    apv(rsb, [[1, nK * H]]), apv(rs, [[1, nK * H]]))
                attn = singles.tile([P, n, K, K, H], BF16, name=f"at_{gi}")
                nc.vector.tensor_mul(
                    apv(attn, [[K * H, nK], [H, K], [1, H]]),
                    apv(es, [[K * H, nK], [H, K], [1, H]]),
                    apv(rsb, [[H, nK], [0, K], [1, H]]))
                w1 = singles.tile([P, n, 3, K, H], BF16, name=f"w1_{gi}")
                nc.vector.tensor_add(
                    apv(w1, [[3 * KH, n], [KH, 3], [1, KH]]),
                    apv(attn, [[K * KH, n], [KH, 3], [1, KH]]),
                    apv(attn, [[K * KH, n], [KH, 3], [1, KH]], 3 * KH))
                w2 = singles.tile([P, n, K, H], BF16, name=f"w2_{gi}")
                nc.vector.tensor_add(
                    apv(w2, [[KH, n], [1, KH]]),
                    apv(w1, [[3 * KH, n], [1, KH]]),
                    apv(w1, [[3 * KH, n], [1, KH]], KH))
                wbar = singles.tile([P, n, K, H], BF16, name=f"wb_{gi}")
                nc.vector.tensor_add(
                    apv(wbar, [[KH, n], [1, KH]]),
                    apv(w2, [[KH, n], [1, KH]]),
                    apv(w1, [[3 * KH, n], [1, KH]], 2 * KH))

                # ---- logits + expert softmax + store for THIS group ----
                lt = singles.tile([P, n, KH, E], BF16, name=f"lt_{gi}")
                nc.vector.tensor_mul(
                    apv(lt, [[KH * E, n], [E, KH], [1, E]]),
                    apv(wbar, [[KH, n], [1, KH], [0, E]]),
                    apv(ve_g, [[KH * E, n], [E, KH], [1, E]]))
                lt1 = singles.tile([P, n, 12, E], BF16, name=f"lt1_{gi}")
                nc.vector.tensor_add(
                    apv(lt1, [[96, n], [1, 96]]),
                    apv(lt, [[KH * E, n], [1, 96]]),
                    apv(lt, [[KH * E, n], [1, 96]], 96))
                lt2 = singles.tile([P, n, 6, E], BF16, name=f"lt2_{gi}")
                nc.vector.tensor_add(
                    apv(lt2, [[48, n], [1, 48]]),
                    apv(lt1, [[96, n], [1, 48]]),
                    apv(lt1, [[96, n], [1, 48]], 48))
                lt3 = singles.tile([P, n, 3, E], BF16, name=f"lt3_{gi}")
                nc.vector.tensor_add(
                    apv(lt3, [[24, n], [1, 24]]),
                    apv(lt2, [[48, n], [1, 24]]),
                    apv(lt2, [[48, n], [1, 24]], 24))
                lt4 = singles.tile([P, n, E], BF16, name=f"lt4_{gi}")
                nc.vector.tensor_add(
                    apv(lt4, [[E, n], [1, E]]),
                    apv(lt3, [[24, n], [1, E]]),
                    apv(lt3, [[24, n], [1, E]], E))
                ltr = singles.tile([P, n, E], F32, name=f"ltr_{gi}")
                nc.vector.tensor_add(
                    apv(ltr, [[E, n], [1, E]]),
                    apv(lt4, [[E, n], [1, E]]),
                    apv(lt3, [[24, n], [1, E]], 2 * E))
                el = singles.tile([P, n, E], F32, name=f"el_{gi}")
                nc.scalar.activation(out=el, in_=ltr, func=AF.Exp)
                zf = singles.tile([P, n], F32, name=f"zf_{gi}")
                nc.vector.reduce_sum(out=zf, in_=el, axis=AX.X)
                rf = singles.tile([P, n], F32, name=f"rf_{gi}")
                nc.vector.reciprocal_approx_fast(rf, zf)
                og = singles.tile([P, n, E], F32, name=f"og_{gi}")
                nc.vector.tensor_mul(og, el, apv(rf, [[1, n], [0, E]]))
                nc.sync.dma_start(out=out[:, t0:t0 + n, :], in_=og)


    nc.finalize()
    return nc


_NC = None


def _get_module():
    global _NC
    if _NC is None:
        _NC = _build_module()
    return _NC


def _host_prep(x, proj_w, proj_b, in_proj_w, in_proj_b, out_w, out_b, fc_w, fc_b):
    scale = np.float32(1.0 / np.sqrt(HD))
    w_eff = (in_proj_w @ proj_w).astype(np.float32)          # [96, 64]
    b_eff = (in_proj_w @ proj_b + in_proj_b).astype(np.float32)
    w_eff[0:D] *= scale
    b_eff[0:D] *= scale

    g = (fc_w @ out_w / np.float32(K)).astype(np.float32)    # [8, 32]
    g_b = (fc_w @ out_b + fc_b).astype(np.float32)
    # fold out-proj+fc into per-(step,head) partial-logit weights:
    # ve_(h,e) = G[e, 8h:8h+8] @ v_(h,:) + g_b[e]/24; since
    # sum_{j,h} wbar[j,h] = K*H = 24, logits = sum_{j,h} wbar*ve exactly.
    for h in range(H):
        gh = g[:, h * HD:(h + 1) * HD]                       # [8, 8]
        rows = slice(2 * D + h * HD, 2 * D + (h + 1) * HD)
        w_eff[rows] = gh @ w_eff[rows]
        b_eff[rows] = gh @ b_eff[rows] + g_b / np.float32(K * H)

    wa = np.concatenate([w_eff.T, b_eff[None, :]], axis=0)   # [65, 96]
    wa = np.ascontiguousarray(wa).astype(NP_BF16)

    # x: [B, T, N, C] -> last K steps -> per-core [65, K, S_PAD] feature-major
    xk = x[:, T - K:, :, :]                                  # [B, K, N, C]
    in_maps = []
    for core in range(NCORES):
        xc = xk[core * B_SH:(core + 1) * B_SH]               # [8, K, N, C]
        # -> [C, K, b, N] -> [C, K, S]
        xc = np.transpose(xc, (3, 1, 0, 2)).reshape(C, K, S)
        xtc = np.ones((CA, K, S_PAD), dtype=NP_BF16)
        xtc[0:C, :, 0:S] = xc.astype(NP_BF16)
        xtc[0:C, :, S:] = 0
        in_maps.append({"xt": xtc, "wa": wa})
    return in_maps


def kernel(x, proj_w, proj_b, in_proj_w, in_proj_b, out_w, out_b, fc_w, fc_b,
           _trace=False):
    in_maps = _host_prep(np.asarray(x, dtype=np.float32),
                         np.asarray(proj_w, dtype=np.float32),
                         np.asarray(proj_b, dtype=np.float32),
                         np.asarray(in_proj_w, dtype=np.float32),
                         np.asarray(in_proj_b, dtype=np.float32),
                         np.asarray(out_w, dtype=np.float32),
                         np.asarray(out_b, dtype=np.float32),
                         np.asarray(fc_w, dtype=np.float32),
                         np.asarray(fc_b, dtype=np.float32))
    nc = _get_module()
    res = run_bass_kernel_spmd(nc, in_maps, core_ids=list(range(NCORES)),
                               trace=_trace)
    outs = []
    for core in range(NCORES):
        oc = res.results[core]["out"]                        # [P, NT, E]
        oc = oc.transpose(1, 0, 2).reshape(S_PAD, E)[:S]
        oc = oc.reshape(B_SH, NTOK, E)
        outs.append(oc)
    full = np.concatenate(outs, axis=0)                      # [64, 207, 8]
    if _trace:
        kernel._last_exec_time_ns = res.exec_time_ns
        kernel._last_profile = res.profile_json
    return full.astype(np.float32)



# revision 37
# speedup vs baseline: 1.1602x; 1.1602x over previous
"""Trainium2 Bass kernel for nn_AttentiveGatingv2 (moe_routing).

Reference computation (shapes hardcoded):
  x: [64, 96, 207, 64] -> take last 6 timesteps -> per-(b,n) token:
  z = proj(x_k); qkv = in_proj(z); 4-head attention over the 6 steps;
  out-proj; mean over steps; fc to 8 experts; softmax -> [64, 207, 8].

Host-side algebraic fusion (verified vs reference):
  W_eff = in_proj_w @ proj_w  (96x64), b_eff = in_proj_w@proj_b + in_proj_b
  (q-rows pre-scaled by 1/sqrt(8)); since mean-over-steps commutes with the
  linear out-proj/fc, post-attention collapses to
  logits = G @ (sum_j wbar_j * v_j) + g_b  with  G = fc_w@out_w/6,
  g_b = fc_w@out_b + fc_b,  wbar_j = sum_i softmax_j(scores)_ij.

Layout strategy: host pre-slices the 6 needed timesteps (1/16 of x), casts
to bf16, packs them feature-major with an appended ones-row so the single
PE matmul  qkv[tokens,96] = x_aug[65,tokens].T @ W_aug[65,96]  lands
token-major in PSUM (fp32) with bias included.  Attention math runs on
VectorE in bf16 (validated ~5e-4 rel-to-max on the final softmax output),
with 128-token tiles processed in groups of 4 so the small softmax/context
ops amortize instruction overhead; expert-logit matmuls run per pair of
tiles through one PE transpose + a block-diagonal G.  8 NeuronCores
data-parallel over batch; no cross-device communication.

Measured on trn2 (8 cores, via axon): HW exec ~58.5us/core, rel err 5.4e-4.
Progression: 160us (first correct fp32) -> 90 (drop serializing hacks) ->
73 (bf16) -> 62 (pair batching) -> 58.5 (quad batching, load stagger,
scalar-queue const loads, deeper work pool). GQ=6 measured
worse (60.4us: batched ops stall behind 6 evacuations), so GQ=4 stands.
"""

import numpy as np
import ml_dtypes

import concourse.bass as bass
import concourse.mybir as mybir
import concourse.tile as tile
from concourse.bacc import Bacc
from concourse.bass_utils import run_bass_kernel_spmd

F32 = mybir.dt.float32
BF16 = mybir.dt.bfloat16
NP_BF16 = ml_dtypes.bfloat16

# problem dims
B, T, NTOK, C = 64, 96, 207, 64
D, H, HD, K = 32, 4, 8, 6
E = 8
NCORES = 8

# per-core dims
B_SH = B // NCORES            # 8
S = B_SH * NTOK               # 1656 tokens per core
P = 128
NT = (S + P - 1) // P         # 13 tiles
S_PAD = NT * P                # 1664
CA = C + 1                    # 65: channels + ones row
E3 = 3 * D                    # 96
KK = K * K                    # 36
DA = D + 1                    # 33


def _build_module():
    nc = Bacc()

    xt = nc.dram_tensor("xt", [CA, K, S_PAD], BF16, kind="ExternalInput")
    # wa columns: q (32, pre-scaled) | k (32) | ve (32 = 4 heads x 8 experts)
    # where ve_(h,e) = G[e, 8h:8h+8] @ v_(h,:) + g_b[e]/24, so
    # logits = sum_{j,h} wbar[j,h] * ve[j,h,:] exactly (sum wbar = 24).
    wa = nc.dram_tensor("wa", [CA, E3], BF16, kind="ExternalInput")
    # out[p, t, e]: token (t*128+p); host reassembles.
    out = nc.dram_tensor("out", [P, NT, E], F32, kind="ExternalOutput")

    AF = mybir.ActivationFunctionType
    AX = mybir.AxisListType

    def apv(t, dims, extra_offset=0):
        # custom AP over tile t: keep t's partition dim, replace free dims
        return bass.AP(
            tensor=t.tensor,
            offset=t.offset + extra_offset,
            ap=[list(t.ap[0])] + [list(d) for d in dims],
        )

    KH = K * H
    # two groups, each carried to completion (through its own logits +
    # expert-softmax + store): group 0's tail overlaps group 1's compute,
    # so the end-of-kernel serial chain is only group 1's half
    GROUPS = [(0, 7), (7, 6)]

    with tile.TileContext(nc) as tc:
        with (
            tc.tile_pool(name="singles", bufs=1) as singles,
            tc.tile_pool(name="xload", bufs=1) as xload,
            tc.tile_pool(name="psum", bufs=4, space="PSUM") as psum,
        ):
            # wa on the scalar queue: overlaps the x tile-0 load on sync
            wa_sb = singles.tile([CA, E3], BF16)
            nc.scalar.dma_start(out=wa_sb, in_=wa[:, :])

            # early tiles land ASAP: tile 0 split across BOTH hw queues,
            # tiles 1-3 alternate queues; the rest in two chunks
            load_plan = [(0, 1), (1, 1), (2, 1), (3, 1), (4, 4), (8, 5)]
            xt_sb = [None] * NT
            xld = []
            for li, (t0, n) in enumerate(load_plan):
                xc_sb = xload.tile([CA, K, n * P], BF16, name=f"xc{t0}",
                                   tag=f"xc{t0}")
                if li == 0:
                    nc.sync.dma_start(out=xc_sb[:, 0:3, :],
                                      in_=xt[:, 0:3, t0 * P:(t0 + 1) * P])
                    d = nc.scalar.dma_start(
                        out=xc_sb[:, 3:K, :],
                        in_=xt[:, 3:K, t0 * P:(t0 + 1) * P])
                else:
                    eng = nc.scalar if li in (2, 5) else nc.sync
                    d = eng.dma_start(out=xc_sb,
                                      in_=xt[:, :, t0 * P:(t0 + n) * P])
                xld.append(d)
                for u in range(n):
                    xt_sb[t0 + u] = xc_sb[:, :, u * P:(u + 1) * P]

            qkv_first = []
            for gi, (t0, n) in enumerate(GROUPS):
                tmp_g = singles.tile([P, n, KK, D], BF16, name=f"tmp{gi}")
                ve_g = singles.tile([P, n, K, D], BF16, name=f"ve{gi}")
                for u in range(n):
                    t = t0 + u
                    # [P, 8, 128] = exactly 2 PSUM banks (bank-aligned slots)
                    qkv_ps = psum.tile([P, 8, 128], F32, tag="qkv_ps",
                                       bufs=4, name="qkv_ps")
                    for i in range(K):
                        mm = nc.tensor.matmul(
                            out=qkv_ps[:, i, 0:E3],
                            lhsT=xt_sb[t][:, i, :],
                            rhs=wa_sb[:, :],
                            start=True,
                            stop=True,
                        )
                        if i == 0:
                            qkv_first.append(mm)

                    # evacuate as bf16 on ScalarE; per-tile qk tile keeps the
                    # evac->tmp dependency precise
                    qk_sb = singles.tile([P, K, 2 * D], BF16, name=f"qk{t}")
                    nc.scalar.copy(out=qk_sb, in_=qkv_ps[:, 0:K, 0:2 * D])
                    nc.scalar.copy(out=ve_g[:, u],
                                   in_=qkv_ps[:, 0:K, 2 * D:E3])

                    # scores tmp[i,j,(h,c)] = q[i,(hc)] * k[j,(hc)]
                    q_ap = apv(qk_sb, [[2 * D, K], [0, K], [1, D]])
                    k_ap = apv(qk_sb, [[0, K], [2 * D, K], [1, D]], D)
                    tm_out = apv(tmp_g, [[D, KK], [1, D]], u * KK * D)
                    nc.vector.tensor_mul(tm_out, q_ap, k_ap)

                # ---- c-reduction tree batched over the group ----
                aKK = n * KK
                s1 = singles.tile([P, n, KK, H, 4], BF16, name=f"s1_{gi}")
                nc.vector.tensor_add(
                    apv(s1, [[16, aKK], [4, H], [1, 4]]),
                    apv(tmp_g, [[D, aKK], [HD, H], [1, 4]]),
                    apv(tmp_g, [[D, aKK], [HD, H], [1, 4]], 4))
                s2 = singles.tile([P, n, KK, H, 2], BF16, name=f"s2_{gi}")
                nc.vector.tensor_add(
                    apv(s2, [[8, aKK], [2, H], [1, 2]]),
                    apv(s1, [[16, aKK], [4, H], [1, 2]]),
                    apv(s1, [[16, aKK], [4, H], [1, 2]], 2))
                scores = singles.tile([P, n, KK, H], F32, name=f"sc_{gi}")
                nc.vector.tensor_add(
                    apv(scores, [[H, aKK], [1, H]]),
                    apv(s2, [[8, aKK], [2, H]]),
                    apv(s2, [[8, aKK], [2, H]], 1))
                es = singles.tile([P, n, K, K, H], BF16, name=f"es_{gi}")
                nc.scalar.activation(out=es, in_=scores, func=AF.Exp)
                nK = n * K
                zs1 = singles.tile([P, n, K, 3, H], BF16, name=f"zs1_{gi}")
                nc.vector.tensor_add(
                    apv(zs1, [[3 * H, nK], [H, 3], [1, H]]),
                    apv(es, [[K * H, nK], [H, 3], [1, H]]),
                    apv(es, [[K * H, nK], [H, 3], [1, H]], 3 * H))
                zs2 = singles.tile([P, n, K, H], BF16, name=f"zs2_{gi}")
                nc.vector.tensor_add(
                    apv(zs2, [[H, nK], [1, H]]),
                    apv(zs1, [[3 * H, nK], [1, H]]),
                    apv(zs1, [[3 * H, nK], [1, H]], H))
                zs = singles.tile([P, n, K, H], F32, name=f"zs_{gi}")
                nc.vector.tensor_add(
                    apv(zs, [[H, nK], [1, H]]),
                    apv(zs2, [[H, nK], [1, H]]),
                    apv(zs1, [[3 * H, nK], [1, H]], 2 * H))
                rs = singles.tile([P, n, K, H], F32, name=f"rs_{gi}")
                nc.vector.reciprocal_approx_fast(
                    apv(rs, [[1, nK * H]]), apv(zs, [[1, nK * H]]))
                rsb = singles.tile([P, n, K, H], BF16, name=f"rsb_{gi}")
                nc.vector.tensor_copy(
                    apv(rsb, [[1, nK * H]]), apv(rs, [[1, nK * H]]))
                attn = singles.tile([P, n, K, K, H], BF16, name=f"at_{gi}")
                nc.vector.tensor_mul(
                    apv(attn, [[K * H, nK], [H, K], [1, H]]),
                    apv(es, [[K * H, nK], [H, K], [1, H]]),
                    apv(rsb, [[H, nK], [0, K], [1, H]]))
                w1 = singles.tile([P, n, 3, K, H], BF16, name=f"w1_{gi}")
                nc.vector.tensor_add(
                    apv(w1, [[3 * KH, n], [KH, 3], [1, KH]]),
                    apv(attn, [[K * KH, n], [KH, 3], [1, KH]]),
                    apv(attn, [[K * KH, n], [KH, 3], [1, KH]], 3 * KH))
                w2 = singles.tile([P, n, K, H], BF16, name=f"w2_{gi}")
                nc.vector.tensor_add(
                    apv(w2, [[KH, n], [1, KH]]),
                    apv(w1, [[3 * KH, n], [1, KH]]),
                    apv(w1, [[3 * KH, n], [1, KH]], KH))
                wbar = singles.tile([P, n, K, H], BF16, name=f"wb_{gi}")
                nc.vector.tensor_add(
                    apv(wbar, [[KH, n], [1, KH]]),
                    apv(w2, [[KH, n], [1, KH]]),
                    apv(w1, [[3 * KH, n], [1, KH]], 2 * KH))

                # ---- logits + expert softmax + store for THIS group ----
                lt = singles.tile([P, n, KH, E], BF16, name=f"lt_{gi}")
                nc.vector.tensor_mul(
                    apv(lt, [[KH * E, n], [E, KH], [1, E]]),
                    apv(wbar, [[KH, n], [1, KH], [0, E]]),
                    apv(ve_g, [[KH * E, n], [E, KH], [1, E]]))
                lt1 = singles.tile([P, n, 12, E], BF16, name=f"lt1_{gi}")
                nc.vector.tensor_add(
                    apv(lt1, [[96, n], [1, 96]]),
                    apv(lt, [[KH * E, n], [1, 96]]),
                    apv(lt, [[KH * E, n], [1, 96]], 96))
                lt2 = singles.tile([P, n, 6, E], BF16, name=f"lt2_{gi}")
                nc.vector.tensor_add(
                    apv(lt2, [[48, n], [1, 48]]),
                    apv(lt1, [[96, n], [1, 48]]),
                    apv(lt1, [[96, n], [1, 48]], 48))
                lt3 = singles.tile([P, n, 3, E], BF16, name=f"lt3_{gi}")
                nc.vector.tensor_add(
                    apv(lt3, [[24, n], [1, 24]]),
                    apv(lt2, [[48, n], [1, 24]]),
                    apv(lt2, [[48, n], [1, 24]], 24))
                lt4 = singles.tile([P, n, E], BF16, name=f"lt4_{gi}")
                nc.vector.tensor_add(
                    apv(lt4, [[E, n], [1, E]]),
                    apv(lt3, [[24, n], [1, E]]),
                    apv(lt3, [[24, n], [1, E]], E))
                ltr = singles.tile([P, n, E], F32, name=f"ltr_{gi}")
                nc.vector.tensor_add(
                    apv(ltr, [[E, n], [1, E]]),
                    apv(lt4, [[E, n], [1, E]]),
                    apv(lt3, [[24, n], [1, E]], 2 * E))
                el = singles.tile([P, n, E], F32, name=f"el_{gi}")
                nc.scalar.activation(out=el, in_=ltr, func=AF.Exp)
                zf = singles.tile([P, n], F32, name=f"zf_{gi}")
                nc.vector.reduce_sum(out=zf, in_=el, axis=AX.X)
                rf = singles.tile([P, n], F32, name=f"rf_{gi}")
                nc.vector.reciprocal_approx_fast(rf, zf)
                og = singles.tile([P, n, E], F32, name=f"og_{gi}")
                nc.vector.tensor_mul(og, el, apv(rf, [[1, n], [0, E]]))
                nc.sync.dma_start(out=out[:, t0:t0 + n, :], in_=og)


            # chunk loads stagger behind early qkv
            tile.add_dep_helper(xld[4].ins, qkv_first[0].ins,
                                sync=True, reason="load stagger")
            tile.add_dep_helper(xld[5].ins, qkv_first[4].ins,
                                sync=True, reason="load stagger")

    nc.finalize()
    return nc


_NC = None


def _get_module():
    global _NC
    if _NC is None:
        _NC = _build_module()
    return _NC


def _host_prep(x, proj_w, proj_b, in_proj_w, in_proj_b, out_w, out_b, fc_w, fc_b):
    scale = np.float32(1.0 / np.sqrt(HD))
    w_eff = (in_proj_w @ proj_w).astype(np.float32)          # [96, 64]
    b_eff = (in_proj_w @ proj_b + in_proj_b).astype(np.float32)
    w_eff[0:D] *= scale
    b_eff[0:D] *= scale

    g = (fc_w @ out_w / np.float32(K)).astype(np.float32)    # [8, 32]
    g_b = (fc_w @ out_b + fc_b).astype(np.float32)
    # fold out-proj+fc into per-(step,head) partial-logit weights:
    # ve_(h,e) = G[e, 8h:8h+8] @ v_(h,:) + g_b[e]/24; since
    # sum_{j,h} wbar[j,h] = K*H = 24, logits = sum_{j,h} wbar*ve exactly.
    for h in range(H):
        gh = g[:, h * HD:(h + 1) * HD]                       # [8, 8]
        rows = slice(2 * D + h * HD, 2 * D + (h + 1) * HD)
        w_eff[rows] = gh @ w_eff[rows]
        b_eff[rows] = gh @ b_eff[rows] + g_b / np.float32(K * H)

    wa = np.concatenate([w_eff.T, b_eff[None, :]], axis=0)   # [65, 96]
    wa = np.ascontiguousarray(wa).astype(NP_BF16)

    # x: [B, T, N, C] -> last K steps -> per-core [65, K, S_PAD] feature-major
    xk = x[:, T - K:, :, :]                                  # [B, K, N, C]
    in_maps = []
    for core in range(NCORES):
        xc = xk[core * B_SH:(core + 1) * B_SH]               # [8, K, N, C]
        # -> [C, K, b, N] -> [C, K, S]
        xc = np.transpose(xc, (3, 1, 0, 2)).reshape(C, K, S)
        xtc = np.ones((CA, K, S_PAD), dtype=NP_BF16)
        xtc[0:C, :, 0:S] = xc.astype(NP_BF16)
        xtc[0:C, :, S:] = 0
        in_maps.append({"xt": xtc, "wa": wa})
    return in_maps


def kernel(x, proj_w, proj_b, in_proj_w, in_proj_b, out_w, out_b, fc_w, fc_b,
           _trace=False):
    in_maps = _host_prep(np.asarray(x, dtype=np.float32),
                         np.asarray(proj_w, dtype=np.float32),
                         np.asarray(proj_b, dtype=np.float32),
                         np.asarray(in_proj_w, dtype=np.float32),
                         np.asarray(in_proj_b, dtype=np.float32),
                         np.asarray(out_w, dtype=np.float32),
                         np.asarray(out_b, dtype=np.float32),
                         np.asarray(fc_w, dtype=np.float32),
                         np.asarray(fc_b, dtype=np.float32))
    nc = _get_module()
    res = run_bass_kernel_spmd(nc, in_maps, core_ids=list(range(NCORES)),
                               trace=_trace)
    outs = []
    for core in range(NCORES):
        oc = res.results[core]["out"]                        # [P, NT, E]
        oc = oc.transpose(1, 0, 2).reshape(S_PAD, E)[:S]
        oc = oc.reshape(B_SH, NTOK, E)
        outs.append(oc)
    full = np.concatenate(outs, axis=0)                      # [64, 207, 8]
    if _trace:
        kernel._last_exec_time_ns = res.exec_time_ns
        kernel._last_profile = res.profile_json
    return full.astype(np.float32)

